# revision 1
# baseline (speedup 1.0000x reference)
"""CSWin block (B=8,H=W=56,C=256) on 8 trn2 NeuronCores, data-parallel over batch.

Layout strategy (per core, one image of 3136 tokens):
  - LayerNorms computed token-major (tokens on partitions) -> per-token stats are
    per-partition scalars (native tensor_scalar), then PE-transposed to
    channel-major for the matmuls.
  - All on-chip activations feeding matmuls are channel-major [C, T] so matmuls
    contract along partitions with zero transposes between layers.
  - Branch 0 tensors are stored in W-major token order, branch 1 in H-major, so
    every CSWin stripe window is a contiguous 392-token slice for both branches.
  - Attention computes S^T = K^T Q per window (softmax dim on partitions is
    avoided entirely: exp on ACT, row sums come free as a ones-column in the
    transposed-V operand of the O matmul), O accumulates channel-major.
  - LePE depthwise conv runs on the TensorEngine as 9 diag-matrix matmuls with
    shifted access patterns, accumulated in PSUM during the (PSUM-idle) QKV phase.
  - dtypes: bf16 for q/k/v/Et/Vt/lepe/mlp-h + their weights, float32r (TF32-ish)
    for att/proj, fp32 for LN stats, residuals and PSUM.
"""

import sys

sys.path.insert(0, "/opt/trn_rl_repo")

import numpy as np
from contextlib import ExitStack

import concourse.bacc as bacc
import concourse.tile as tile
import concourse.mybir as mybir
from concourse.bass_utils import run_bass_kernel_spmd
from concourse.masks import make_identity

F32 = mybir.dt.float32
F32R = mybir.dt.float32r
BF16 = mybir.dt.bfloat16
AF = mybir.ActivationFunctionType
OP = mybir.AluOpType

B, HH, WW, C = 8, 56, 56, 256
T = HH * WW              # 3136 tokens
NW = 8                   # windows per branch
L = 392                  # tokens per window
NH = 4                   # heads per branch
HD = 32                  # head dim
MC = 98                  # m-chunk (window tokens / 4)
TK = 112                 # token chunk for token-major phases
NTOK = T // TK           # 28
EPS = 1e-5
SM_SCALE = float(HD) ** -0.5

_CACHE = {}


def _rhs_qkv(ln1, bi, kch, t):
    """ln1[kch] is channel-major H-major-token [128, 3136]; returns the rhs AP for
    output N-tile t (392 tokens) in the branch's native token order."""
    if bi == 0:  # W-major order: iterate (w, h)
        return ln1[kch].rearrange("p (h w) -> p w h", h=HH)[:, 7 * t:7 * t + 7, :]
    return ln1[kch][:, L * t:L * (t + 1)]


def _build():
    nc = bacc.Bacc("TRN2", target_bir_lowering=False, debug=False,
                   enable_asserts=False, num_devices=8)

    x_d = nc.dram_tensor("x", [T, C], F32, kind="ExternalInput").ap()
    out_d = nc.dram_tensor("out", [T, C], F32, kind="ExternalOutput").ap()
    wqkv_d = nc.dram_tensor("w_qkv", [C, 3 * C], F32, kind="ExternalInput").ap()
    wproj_d = nc.dram_tensor("w_proj", [C, C], F32, kind="ExternalInput").ap()
    bproj_d = nc.dram_tensor("b_proj", [C], F32, kind="ExternalInput").ap()
    g1_d = nc.dram_tensor("gamma1", [C], F32, kind="ExternalInput").ap()
    be1_d = nc.dram_tensor("beta1", [C], F32, kind="ExternalInput").ap()
    g2_d = nc.dram_tensor("gamma2", [C], F32, kind="ExternalInput").ap()
    be2_d = nc.dram_tensor("beta2", [C], F32, kind="ExternalInput").ap()
    wfc1_d = nc.dram_tensor("w_fc1", [C, 4 * C], F32, kind="ExternalInput").ap()
    bfc1_d = nc.dram_tensor("b_fc1", [4 * C], F32, kind="ExternalInput").ap()
    wfc2_d = nc.dram_tensor("w_fc2", [4 * C, C], F32, kind="ExternalInput").ap()
    bfc2_d = nc.dram_tensor("b_fc2", [C], F32, kind="ExternalInput").ap()
    lw_d = [nc.dram_tensor("lepe_w0", [128, 9], F32, kind="ExternalInput").ap(),
            nc.dram_tensor("lepe_w1", [128, 9], F32, kind="ExternalInput").ap()]
    lb_d = [nc.dram_tensor("lepe_b0", [128], F32, kind="ExternalInput").ap(),
            nc.dram_tensor("lepe_b1", [128], F32, kind="ExternalInput").ap()]

    with tile.TileContext(nc) as tc:
        with ExitStack() as ctx:
            _emit(nc, tc, ctx, locals())
    nc.compile()
    return nc


def _emit(nc, tc, ctx, d):
    x_d, out_d = d["x_d"], d["out_d"]
    lw_d, lb_d = d["lw_d"], d["lb_d"]

    pp = ctx.enter_context(tc.tile_pool(name="pp", bufs=1))
    psmall = ctx.enter_context(tc.tile_pool(name="psmall", bufs=1))

    # ---------------- persistent tensors ----------------
    xtok = pp.tile([TK, NTOK, C], F32, name="xtok")
    x2res = pp.tile([TK, NTOK, C], F32, name="x2res")
    wqkv_b = pp.tile([128, 2, 3 * C], BF16, name="wqkv_b")
    wproj_r = pp.tile([128, 2, C], F32R, name="wproj_r")
    wfc1_b = pp.tile([128, 2, 4 * C], BF16, name="wfc1_b")
    wfc2_b = pp.tile([128, 8, C], BF16, name="wfc2_b")
    lwb = pp.tile([128, 2, 9], BF16, name="lwb")
    g1s = psmall.tile([128, 2], F32, name="g1s")
    b1s = psmall.tile([128, 2], F32, name="b1s")
    g2s = psmall.tile([128, 2], F32, name="g2s")
    b2s = psmall.tile([128, 2], F32, name="b2s")
    bfc1s = psmall.tile([128, 8], F32, name="bfc1s")
    lbs = psmall.tile([128, 2], F32, name="lbs")
    bproj_tm = psmall.tile([TK, C], F32, name="bproj_tm")
    bfc2_tm = psmall.tile([TK, C], F32, name="bfc2_tm")
    id_r = psmall.tile([128, 128], F32R, name="id_r")
    id_b = psmall.tile([128, 128], BF16, name="id_b")
    mv1 = psmall.tile([TK, NTOK, 2], F32, name="mv1")
    rstd1 = psmall.tile([TK, NTOK], F32, name="rstd1")
    std1 = psmall.tile([TK, NTOK], F32, name="std1")
    mv2 = psmall.tile([TK, NTOK, 2], F32, name="mv2")
    rstd2 = psmall.tile([TK, NTOK], F32, name="rstd2")
    std2 = psmall.tile([TK, NTOK], F32, name="std2")
    eps_t = psmall.tile([TK, 1], F32, name="eps_t")
    nc.vector.memset(eps_t, EPS)

    # ---------------- phase A: loads + weight conversion ----------------
    with tc.tile_pool(name="stg", bufs=1) as stg:
        wqkv_f = stg.tile([128, 2, 3 * C], F32, name="wqkv_f")
        wproj_f = stg.tile([128, 2, C], F32, name="wproj_f")
        wfc1_f = stg.tile([128, 2, 4 * C], F32, name="wfc1_f")
        wfc2_f = stg.tile([128, 8, C], F32, name="wfc2_f")
        lw_f = stg.tile([128, 2, 9], F32, name="lw_f")

        nc.sync.dma_start(wqkv_f, d["wqkv_d"].rearrange("(a p) n -> p a n", p=128))
        nc.sync.dma_start(wproj_f, d["wproj_d"].rearrange("(a p) n -> p a n", p=128))
        nc.sync.dma_start(wfc1_f, d["wfc1_d"].rearrange("(a p) n -> p a n", p=128))
        nc.sync.dma_start(wfc2_f, d["wfc2_d"].rearrange("(a p) n -> p a n", p=128))
        for bi in range(2):
            nc.sync.dma_start(lw_f[:, bi, :], lw_d[bi])
            nc.sync.dma_start(lbs[:, bi:bi + 1], lb_d[bi].unsqueeze(1))
        nc.sync.dma_start(g1s, d["g1_d"].rearrange("(a p) -> p a", p=128))
        nc.sync.dma_start(b1s, d["be1_d"].rearrange("(a p) -> p a", p=128))
        nc.sync.dma_start(g2s, d["g2_d"].rearrange("(a p) -> p a", p=128))
        nc.sync.dma_start(b2s, d["be2_d"].rearrange("(a p) -> p a", p=128))
        nc.sync.dma_start(bfc1s, d["bfc1_d"].rearrange("(a p) -> p a", p=128))
        nc.sync.dma_start(bproj_tm, d["bproj_d"].unsqueeze(0).broadcast_to([TK, C]))
        nc.sync.dma_start(bfc2_tm, d["bfc2_d"].unsqueeze(0).broadcast_to([TK, C]))

        nc.vector.tensor_copy(wqkv_b, wqkv_f)
        nc.vector.tensor_copy(wproj_r, wproj_f)
        nc.vector.tensor_copy(wfc1_b, wfc1_f)
        nc.vector.tensor_copy(wfc2_b, wfc2_f)
        nc.vector.tensor_copy(lwb, lw_f)
        id_f = stg.tile([128, 128], F32, name="id_f")
        make_identity(nc, id_f)
        nc.vector.tensor_copy(id_r, id_f)
        nc.vector.tensor_copy(id_b, id_f)

        # input: token-major [112, 28, 256]; 4 DMAs for queue parallelism
        xsrc = x_d.rearrange("(i p) c -> p i c", p=TK)
        for j in range(4):
            nc.sync.dma_start(xtok[:, 7 * j:7 * j + 7, :], xsrc[:, 7 * j:7 * j + 7, :])

    # ---------------- LN (token-major) helper ----------------
    def layer_norm(src, mv, stdt, rstd, gs, bs, dst, lnp, tpp):
        """src: [TK, NTOK, C] f32; dst: 2 channel-major [128, T] bf16 tiles."""
        for i in range(NTOK):
            st = lnp.tile([TK, 6], F32, name="bnst", tag="bnst")
            nc.vector.bn_stats(st, src[:, i, :])
            nc.vector.bn_aggr(mv[:, i, :], st)
        nc.scalar.activation(stdt, mv[:, :, 1], AF.Sqrt, bias=eps_t)
        nc.vector.reciprocal(rstd, stdt)
        for g in range(7):
            lnt = []
            for j in range(4):
                i = 4 * g + j
                lt = lnp.tile([TK, C], F32R, name="lnt", tag="lnt")
                nc.vector.tensor_scalar(
                    out=lt, in0=src[:, i, :],
                    scalar1=mv[:, i, 0:1], scalar2=rstd[:, i:i + 1],
                    op0=OP.subtract, op1=OP.mult)
                lnt.append(lt)
            for c in range(2):
                tp = tpp.tile([128, 4 * TK], F32R, name="lntp", tag="lntp")
                for j in range(4):
                    nc.tensor.transpose(tp[:, TK * j:TK * (j + 1)],
                                        lnt[j][:, 128 * c:128 * (c + 1)],
                                        id_r[0:TK, 0:TK])
                nc.scalar.activation(dst[c][:, 4 * TK * g:4 * TK * (g + 1)], tp,
                                     AF.Identity, bias=bs[:, c:c + 1],
                                     scale=gs[:, c:c + 1])

    # ---------------- attention-lifetime tensors ----------------
    actx = ExitStack()
    attn_pool = actx.enter_context(tc.tile_pool(name="attn_pool", bufs=1))
    qc = [attn_pool.tile([128, T], BF16, name=f"qc{b}") for b in range(2)]
    kc = [attn_pool.tile([128, T], BF16, name=f"kc{b}") for b in range(2)]
    vtb = [attn_pool.tile([MC, NW, 4, 4 * 33], BF16, name=f"vtb{b}") for b in range(2)]
    lepe_sb = [attn_pool.tile([128, T], BF16, name=f"lepe{b}") for b in range(2)]
    att_pool = actx.enter_context(tc.tile_pool(name="att_pool", bufs=1))
    att = [att_pool.tile([128, T], F32R, name=f"att{b}") for b in range(2)]

    # Optional in-NEFF repetition loop for wall-clock timing (BASS_KERNEL_ITERS>1)
    import os as _os
    _iters = int(_os.environ.get("BASS_KERNEL_ITERS", "1"))
    _skip = _os.environ.get("BASS_KERNEL_SKIP", "")
    loop_cm = tc.For_i(0, _iters, 1) if _iters > 1 else None
    if loop_cm is not None:
        ctx.enter_context(loop_cm)

    # ---------------- phase B: LN1 ----------------
    with tc.tile_pool(name="ln1cm", bufs=1) as lncm:
        ln1 = [lncm.tile([128, T], BF16, name=f"ln1_{c}") for c in range(2)]
        with tc.tile_pool(name="lnp1", bufs=8) as lnp, \
             tc.tile_pool(name="tpp1", bufs=2, space="PSUM") as tpp:
            layer_norm(xtok, mv1, std1, rstd1, g1s, b1s, ln1, lnp, tpp)

        # ---------------- phase C: QKV + V-transpose + LePE ----------------
        with tc.tile_pool(name="vcp", bufs=1) as vcp, \
             tc.tile_pool(name="qkvps", bufs=2, space="PSUM") as qkvps, \
             tc.tile_pool(name="vtps", bufs=2, space="PSUM") as vtps, \
             tc.tile_pool(name="lpps", bufs=2, space="PSUM") as lpps, \
             tc.tile_pool(name="dgp", bufs=2) as dgp:
            vc = [vcp.tile([128, T], BF16, name=f"vc{b}") for b in range(2)]
            # chunk name -> (branch, wqkv col range, destination)
            chunks = [("v0", 0, 512, vc[0]), ("v1", 1, 640, vc[1]),
                      ("q0", 0, 0, qc[0]), ("k0", 0, 256, kc[0]),
                      ("q1", 1, 128, qc[1]), ("k1", 1, 384, kc[1])]
            if "C" in _skip:
                chunks = []
            for ci, (nm, bi, c0, dst) in enumerate(chunks):
                for t2 in range(NW // 2):
                    pt = qkvps.tile([128, 1024], F32, name="qkvt", tag="qkvt")
                    for half in range(2):
                        t = 2 * t2 + half
                        for kch in range(2):
                            nc.tensor.matmul(pt[:, 512 * half:512 * half + L],
                                             wqkv_b[:, kch, c0:c0 + 128],
                                             _rhs_qkv(ln1, bi, kch, t),
                                             start=(kch == 0), stop=(kch == 1))
                    dstap = dst[:, 2 * L * t2:2 * L * (t2 + 1)].rearrange(
                        "p (a x) -> p a x", a=2)
                    srcap = pt.rearrange("p (a x) -> p a x", a=2)[:, :, 0:L]
                    if (t2 + ci) % 2 == 0:
                        nc.scalar.copy(dstap, srcap)
                    else:
                        nc.vector.tensor_copy(dstap, srcap)

            for bi in (range(2) if "C" not in _skip else ()):
                # V transposes -> [token, ch] with interleaved ones columns
                for w2 in range(NW // 2):
                    vt = vtps.tile([MC, 2, 512], BF16, name="vtt", tag="vtt")
                    for half in range(2):
                        w = 2 * w2 + half
                        for m in range(4):
                            nc.tensor.transpose(
                                vt[:, half, 128 * m:128 * (m + 1)],
                                vc[bi][:, L * w + MC * m:L * w + MC * (m + 1)],
                                id_b)
                    dstv = vtb[bi][:, 2 * w2:2 * w2 + 2].rearrange(
                        "p a m (h e) -> p a m h e", e=33)
                    nc.vector.tensor_copy(
                        dstv[:, :, :, :, 0:32],
                        vt.rearrange("p a (m h e) -> p a m h e", m=4, h=4))
                    for half in range(2):
                        nc.vector.memset(dstv[:, half, :, :, 32:33], 1.0)
                # LePE: 9 diag matmuls per window
                dg = dgp.tile([128, 9, 128], BF16, name="diag", tag="diag")
                for tx in (-1, 0, 1):
                    for ty in (-1, 0, 1):
                        wi = ((ty + 1) * 3 + (tx + 1)) if bi == 0 else ((tx + 1) * 3 + (ty + 1))
                        nc.gpsimd.affine_select(
                            out=dg[:, wi, :],
                            in_=lwb[:, bi, wi:wi + 1].broadcast_to([128, 128]),
                            compare_op=OP.is_equal, fill=0.0, base=0,
                            pattern=[[-1, 128]], channel_multiplier=1)
                taps = [(0, 0)] + [(tx, ty) for tx in (-1, 0, 1) for ty in (-1, 0, 1)
                                   if (tx, ty) != (0, 0)]
                for w in range(NW):
                    lp = lpps.tile([128, L], F32, name="lpt", tag="lpt")
                    lpv = lp.rearrange("p (x y) -> p x y", x=7)
                    vcv = vc[bi][:, L * w:L * (w + 1)].rearrange("p (x y) -> p x y", x=7)
                    for ti, (tx, ty) in enumerate(taps):
                        wi = ((ty + 1) * 3 + (tx + 1)) if bi == 0 else ((tx + 1) * 3 + (ty + 1))
                        xo0, xo1 = max(0, -tx), 7 - max(0, tx)
                        yo0, yo1 = max(0, -ty), HH - max(0, ty)
                        nc.tensor.matmul(
                            lpv[:, xo0:xo1, yo0:yo1], dg[:, wi, :],
                            vcv[:, xo0 + tx:xo1 + tx, yo0 + ty:yo1 + ty],
                            start=(ti == 0), stop=(ti == 8))
                    nc.scalar.activation(lepe_sb[bi][:, L * w:L * (w + 1)], lp,
                                         AF.Identity, bias=lbs[:, bi:bi + 1])

    # ---------------- phase D: windowed attention ----------------
    with tc.tile_pool(name="etp", bufs=2) as etp, \
         tc.tile_pool(name="sps", bufs=1, space="PSUM") as sps, \
         tc.tile_pool(name="ops", bufs=1, space="PSUM") as ops, \
         tc.tile_pool(name="rrp", bufs=2) as rrp, \
         tc.tile_pool(name="rbp", bufs=2) as rbp:
        for bi in (range(2) if "D" not in _skip else ()):
            for w in range(NW):
                et = etp.tile([MC, 4, 4, L], BF16, name="et", tag="et")
                ohalf = [ops.tile([33, 1024], F32, name=f"opst{z}", tag=f"opst{z}")
                         for z in range(2)]
                for m in range(4):
                    sp = sps.tile([MC, 2048], F32, name="spst", tag="spst")
                    for h in range(NH):
                        nc.tensor.matmul(
                            sp[:, 512 * h:512 * h + L],
                            kc[bi][32 * h:32 * (h + 1), L * w + MC * m:L * w + MC * (m + 1)],
                            qc[bi][32 * h:32 * (h + 1), L * w:L * (w + 1)],
                            start=True, stop=True, tile_position=(32 * h, 0))
                    nc.scalar.activation(
                        et[:, m, :, :],
                        sp.rearrange("p (h x) -> p h x", h=4)[:, :, 0:L],
                        AF.Exp, scale=SM_SCALE)
                    for h in range(NH):
                        nc.tensor.matmul(
                            ohalf[h // 2][:, 512 * (h % 2):512 * (h % 2) + L],
                            vtb[bi][:, w, m, 33 * h:33 * (h + 1)],
                            et[:, m, h, :],
                            start=(m == 0), stop=(m == 3))
                # att[0] is stored H-major (so proj lhsT slices are contiguous);
                # branch-0 windows therefore write through a strided [w, h] view.
                if bi == 0:
                    attw = att[0].rearrange("p (h w) -> p w h", h=HH)[:, 7 * w:7 * w + 7, :]
                else:
                    attw = att[1][:, L * w:L * (w + 1)].rearrange("p (a b) -> p a b", a=7)
                lpw = lepe_sb[bi][:, L * w:L * (w + 1)].rearrange("p (a b) -> p a b", a=7)
                for z in range(2):
                    op_ = ohalf[z]
                    rr = rrp.tile([1, 2, L], F32, name="rr", tag="rr")
                    nc.vector.reciprocal(
                        rr, op_[32:33, :].rearrange("p (h x) -> p h x", h=2)[:, :, 0:L])
                    rb = rbp.tile([32, 2, L], F32, name="rb", tag="rb")
                    nc.gpsimd.partition_broadcast(rb, rr)
                    for hh in range(2):
                        h = 2 * z + hh
                        nc.vector.tensor_tensor(
                            out=attw[32 * h:32 * (h + 1)],
                            in0=op_[0:32, 512 * hh:512 * hh + L].rearrange(
                                "p (a b) -> p a b", a=7),
                            in1=rb[:, hh, :].rearrange("p (a b) -> p a b", a=7),
                            op=OP.mult)
                nc.vector.tensor_tensor(out=attw, in0=attw, in1=lpw, op=OP.add)

    # ---------------- phase E: proj + residual (token-major out) ----------------
    # xtok += b_proj (broadcast) so the proj evict is a single fused add
    if "E" not in _skip:
        nc.gpsimd.tensor_tensor(
            out=xtok, in0=xtok,
            in1=bproj_tm.unsqueeze(1).broadcast_to([TK, NTOK, C]),
            op=OP.add)
    with tc.tile_pool(name="prps", bufs=4, space="PSUM") as prps:
        for i2 in (range(NTOK // 2) if "E" not in _skip else ()):
            pt = prps.tile([TK, 2, C], F32, name="prt", tag="prt")
            for half in range(2):
                i = 2 * i2 + half
                for kch in range(2):
                    lhs = att[kch][:, TK * i:TK * (i + 1)]
                    nc.tensor.matmul(pt[:, half, :], lhs, wproj_r[:, kch, :],
                                     start=(kch == 0), stop=(kch == 1))
            nc.vector.scalar_tensor_tensor(
                out=x2res[:, 2 * i2:2 * i2 + 2, :], in0=pt, scalar=1.0,
                in1=xtok[:, 2 * i2:2 * i2 + 2, :], op0=OP.mult, op1=OP.add)

    actx.close()

    # ---------------- phase F: LN2 + MLP ----------------
    with tc.tile_pool(name="lnp2", bufs=8) as lnp2, \
         tc.tile_pool(name="tpp2", bufs=2, space="PSUM") as tpp2, \
         tc.tile_pool(name="mlp", bufs=1) as mlp:
        ln2 = [mlp.tile([128, T], BF16, name=f"ln2_{c}") for c in range(2)]
        if "F" not in _skip:
            layer_norm(x2res, mv2, std2, rstd2, g2s, b2s, ln2, lnp2, tpp2)

        h_sb = mlp.tile([128, 8, T], BF16, name="h_sb")
        with tc.tile_pool(name="f1ps", bufs=3, space="PSUM") as f1ps:
            for m8 in (range(8) if "G" not in _skip else ()):
                for tp2 in range(NW // 2):
                    pt = f1ps.tile([128, 1024], F32, name="f1t", tag="f1t")
                    for half in range(2):
                        t = 2 * tp2 + half
                        for kch in range(2):
                            nc.tensor.matmul(pt[:, 512 * half:512 * half + L],
                                             wfc1_b[:, kch, 128 * m8:128 * (m8 + 1)],
                                             ln2[kch][:, L * t:L * (t + 1)],
                                             start=(kch == 0), stop=(kch == 1))
                    nc.scalar.activation(
                        h_sb[:, m8, 2 * L * tp2:2 * L * (tp2 + 1)].rearrange(
                            "p (a x) -> p a x", a=2),
                        pt.rearrange("p (a x) -> p a x", a=2)[:, :, 0:L],
                        AF.Gelu, bias=bfc1s[:, m8:m8 + 1])

        # x2res += b_fc2 (broadcast) after LN2 consumed raw x2res
        if "G" not in _skip:
            nc.gpsimd.tensor_tensor(
                out=x2res, in0=x2res,
                in1=bfc2_tm.unsqueeze(1).broadcast_to([TK, NTOK, C]),
                op=OP.add)
        with tc.tile_pool(name="f2ps", bufs=4, space="PSUM") as f2ps, \
             tc.tile_pool(name="otp", bufs=4) as otp:
            for i2 in (range(NTOK // 2) if "G" not in _skip else ()):
                pt = f2ps.tile([TK, 2, C], F32, name="f2t", tag="f2t")
                for half in range(2):
                    i = 2 * i2 + half
                    for k8 in range(8):
                        nc.tensor.matmul(pt[:, half, :], h_sb[:, k8, TK * i:TK * (i + 1)],
                                         wfc2_b[:, k8, :],
                                         start=(k8 == 0), stop=(k8 == 7))
                ot = otp.tile([TK, 2, C], F32, name="ot", tag="ot")
                nc.vector.scalar_tensor_tensor(
                    out=ot, in0=pt, scalar=1.0, in1=x2res[:, 2 * i2:2 * i2 + 2, :],
                    op0=OP.mult, op1=OP.add)
                eng = nc.sync if i2 % 2 == 0 else nc.scalar
                eng.dma_start(
                    out_d[2 * TK * i2:2 * TK * (i2 + 1), :].rearrange(
                        "(a p) c -> p a c", p=TK),
                    ot)


def kernel(**inputs):
    if "nc" not in _CACHE:
        _CACHE["nc"] = _build()
    nc = _CACHE["nc"]

    x = np.asarray(inputs["x"], dtype=np.float32)          # [8, 56, 56, 256]
    base = {
        "w_qkv": np.asarray(inputs["w_qkv"], np.float32),
        "w_proj": np.asarray(inputs["w_proj"], np.float32),
        "b_proj": np.asarray(inputs["b_proj"], np.float32),
        "gamma1": np.asarray(inputs["gamma1"], np.float32),
        "beta1": np.asarray(inputs["beta1"], np.float32),
        "gamma2": np.asarray(inputs["gamma2"], np.float32),
        "beta2": np.asarray(inputs["beta2"], np.float32),
        "w_fc1": np.asarray(inputs["w_fc1"], np.float32),
        "b_fc1": np.asarray(inputs["b_fc1"], np.float32),
        "w_fc2": np.asarray(inputs["w_fc2"], np.float32),
        "b_fc2": np.asarray(inputs["b_fc2"], np.float32),
        "lepe_w0": np.asarray(inputs["lepe_w0"], np.float32).reshape(128, 9),
        "lepe_w1": np.asarray(inputs["lepe_w1"], np.float32).reshape(128, 9),
        "lepe_b0": np.asarray(inputs["lepe_b0"], np.float32),
        "lepe_b1": np.asarray(inputs["lepe_b1"], np.float32),
    }
    in_maps = [{**base, "x": np.ascontiguousarray(x[i].reshape(T, C))}
               for i in range(B)]
    import os
    trace = bool(int(os.environ.get("BASS_KERNEL_TRACE", "0")))
    res = run_bass_kernel_spmd(nc, in_maps, core_ids=list(range(B)), trace=trace)
    _CACHE["last_results"] = res
    out = np.stack([res.results[i]["out"] for i in range(B)])
    return out.reshape(B, HH, WW, C)


if __name__ == "__main__":
    rng = np.random.default_rng(0)
    ins = {
        "x": rng.standard_normal((B, HH, WW, C), dtype=np.float32),
        "gamma1": np.ones(C, np.float32), "beta1": np.zeros(C, np.float32),
        "w_qkv": rng.standard_normal((C, 3 * C), dtype=np.float32) * 0.02,
        "lepe_w0": rng.standard_normal((128, 1, 3, 3), dtype=np.float32) * 0.02,
        "lepe_b0": np.zeros(128, np.float32),
        "lepe_w1": rng.standard_normal((128, 1, 3, 3), dtype=np.float32) * 0.02,
        "lepe_b1": np.zeros(128, np.float32),
        "w_proj": rng.standard_normal((C, C), dtype=np.float32) * 0.02,
        "b_proj": np.zeros(C, np.float32),
        "gamma2": np.ones(C, np.float32), "beta2": np.zeros(C, np.float32),
        "w_fc1": rng.standard_normal((C, 4 * C), dtype=np.float32) * 0.02,
        "b_fc1": np.zeros(4 * C, np.float32),
        "w_fc2": rng.standard_normal((4 * C, C), dtype=np.float32) * 0.02,
        "b_fc2": np.zeros(C, np.float32),
    }
    o = kernel(**ins)
    print("ran:", o.shape, o.dtype, float(np.abs(o).max()))



# revision 9
# speedup vs baseline: 1.0188x; 1.0188x over previous
"""CSWin block (B=8,H=W=56,C=256) on 8 trn2 NeuronCores, data-parallel over batch.

v2 layout strategy (per core, one image of 3136 tokens):
  - Residual stream token-major [112, 28, 256] fp32; LN stats token-major,
    LN output channel-major fp8 [128, 2(kch), T] feeding DoubleRow matmuls.
  - fp8e4 DoubleRow (0.5 cyc/output-row, 256-deep contraction per instruction)
    for QKV, fc1, fc2, proj. Biases for proj/fc2 enter PSUM via K=1 ones
    matmuls. b_fc1 rides the GELU activation per-partition.
  - Attention O is computed TRANSPOSED (window q-tokens on partitions) via
    fp8 DoubleRow over k-chunk pairs; a ones-column in V^T produces the
    softmax denominator as a per-partition column -> normalize is a
    per-partition reciprocal + stride-0-broadcast multiply, then one small
    PE transpose per q-chunk returns to channel-major for proj.
  - Branch-0 q/k/v stored W-major so its stripe windows are contiguous;
    PSUM evacuations scatter between H-major and W-major orders.
  - LePE depthwise conv stays on the PE as 9 diag-matmul taps (bf16).
  - exp on the Act engine is the wall (~95us); everything else is spread
    across DVE/Pool/PE underneath it.
"""

import sys

sys.path.insert(0, "/opt/trn_rl_repo")

import numpy as np
from contextlib import ExitStack

import concourse.bacc as bacc
import concourse.tile as tile
import concourse.mybir as mybir
from concourse.bass_utils import run_bass_kernel_spmd
from concourse.masks import make_identity

F32 = mybir.dt.float32
BF16 = mybir.dt.bfloat16
F8 = mybir.dt.float8e4
AF = mybir.ActivationFunctionType
OP = mybir.AluOpType
PM = mybir.MatmulPerfMode

B, HH, WW, C = 8, 56, 56, 256
T = HH * WW              # 3136 tokens
NW = 8                   # windows per branch
L = 392                  # tokens per window
NH = 4                   # heads per branch
HD = 32                  # head dim
TK = 112                 # token chunk for token-major phases
NTOK = T // TK           # 28
MCS = [112, 112, 112, 56]  # window k/q chunk sizes (112*3 + 56 = 392)
MCO = [0, 112, 224, 336]   # their offsets
EPS = 1e-5
SM_SCALE = float(HD) ** -0.5

_CACHE = {}


def _build():
    nc = bacc.Bacc("TRN2", target_bir_lowering=False, debug=False,
                   enable_asserts=False, num_devices=8)

    x_d = nc.dram_tensor("x", [T, C], F32, kind="ExternalInput").ap()
    out_d = nc.dram_tensor("out", [T, C], F32, kind="ExternalOutput").ap()
    wqkv_d = nc.dram_tensor("w_qkv", [C, 3 * C], F32, kind="ExternalInput").ap()
    wproj_d = nc.dram_tensor("w_proj", [C, C], F32, kind="ExternalInput").ap()
    bproj_d = nc.dram_tensor("b_proj", [C], F32, kind="ExternalInput").ap()
    g1_d = nc.dram_tensor("gamma1", [C], F32, kind="ExternalInput").ap()
    be1_d = nc.dram_tensor("beta1", [C], F32, kind="ExternalInput").ap()
    g2_d = nc.dram_tensor("gamma2", [C], F32, kind="ExternalInput").ap()
    be2_d = nc.dram_tensor("beta2", [C], F32, kind="ExternalInput").ap()
    wfc1_d = nc.dram_tensor("w_fc1", [C, 4 * C], F32, kind="ExternalInput").ap()
    bfc1_d = nc.dram_tensor("b_fc1", [4 * C], F32, kind="ExternalInput").ap()
    wfc2_d = nc.dram_tensor("w_fc2", [4 * C, C], F32, kind="ExternalInput").ap()
    bfc2_d = nc.dram_tensor("b_fc2", [C], F32, kind="ExternalInput").ap()
    lw_d = [nc.dram_tensor("lepe_w0", [128, 9], F32, kind="ExternalInput").ap(),
            nc.dram_tensor("lepe_w1", [128, 9], F32, kind="ExternalInput").ap()]
    lb_d = [nc.dram_tensor("lepe_b0", [128], F32, kind="ExternalInput").ap(),
            nc.dram_tensor("lepe_b1", [128], F32, kind="ExternalInput").ap()]

    with tile.TileContext(nc) as tc:
        with ExitStack() as ctx:
            _emit(nc, tc, ctx, locals())
    nc.compile()
    return nc


def _emit(nc, tc, ctx, d):
    x_d, out_d = d["x_d"], d["out_d"]
    lw_d, lb_d = d["lw_d"], d["lb_d"]

    pp = ctx.enter_context(tc.tile_pool(name="pp", bufs=1))
    psmall = ctx.enter_context(tc.tile_pool(name="psmall", bufs=1))

    # ---------------- persistent tensors ----------------
    xtok = pp.tile([TK, NTOK, C], F32, name="xtok")
    x2res = pp.tile([TK, NTOK, C], F32, name="x2res")
    wqkv8 = pp.tile([128, 2, 3 * C], F8, name="wqkv8")
    wproj8 = pp.tile([128, 2, C], F8, name="wproj8")
    wfc18 = pp.tile([128, 2, 4 * C], F8, name="wfc18")
    wfc28 = pp.tile([128, 8, C], F8, name="wfc28")
    lwb = pp.tile([128, 2, 9], BF16, name="lwb")
    g1s = psmall.tile([128, 2], F32, name="g1s")
    b1s = psmall.tile([128, 2], F32, name="b1s")
    g2s = psmall.tile([128, 2], F32, name="g2s")
    b2s = psmall.tile([128, 2], F32, name="b2s")
    bfc1s = psmall.tile([128, 8], F32, name="bfc1s")
    lbs = psmall.tile([128, 2], F32, name="lbs")
    bprow = psmall.tile([1, C], BF16, name="bprow")
    bf2row = psmall.tile([1, C], BF16, name="bf2row")
    ones1 = psmall.tile([1, TK], BF16, name="ones1")
    id_b = psmall.tile([128, 128], BF16, name="id_b")
    mv1 = psmall.tile([TK, NTOK, 2], F32, name="mv1")
    rstd1 = psmall.tile([TK, NTOK], F32, name="rstd1")
    std1 = psmall.tile([TK, NTOK], F32, name="std1")
    mv2 = psmall.tile([TK, NTOK, 2], F32, name="mv2")
    rstd2 = psmall.tile([TK, NTOK], F32, name="rstd2")
    std2 = psmall.tile([TK, NTOK], F32, name="std2")
    eps_t = psmall.tile([TK, 1], F32, name="eps_t")
    nc.vector.memset(eps_t, EPS)
    nc.vector.memset(ones1, 1.0)

    # ---------------- phase A: loads + weight conversion ----------------
    with tc.tile_pool(name="stg", bufs=1) as stg:
        wqkv_f = stg.tile([128, 2, 3 * C], F32, name="wqkv_f")
        wproj_f = stg.tile([128, 2, C], F32, name="wproj_f")
        wfc1_f = stg.tile([128, 2, 4 * C], F32, name="wfc1_f")
        wfc2_f = stg.tile([128, 8, C], F32, name="wfc2_f")
        lw_f = stg.tile([128, 2, 9], F32, name="lw_f")
        brow_f = stg.tile([1, 2, C], F32, name="brow_f")

        nc.sync.dma_start(wqkv_f, d["wqkv_d"].rearrange("(a p) n -> p a n", p=128))
        nc.sync.dma_start(wproj_f, d["wproj_d"].rearrange("(a p) n -> p a n", p=128))
        nc.sync.dma_start(wfc1_f, d["wfc1_d"].rearrange("(a p) n -> p a n", p=128))
        nc.sync.dma_start(wfc2_f, d["wfc2_d"].rearrange("(a p) n -> p a n", p=128))
        for bi in range(2):
            nc.sync.dma_start(lw_f[:, bi, :], lw_d[bi])
            nc.sync.dma_start(lbs[:, bi:bi + 1], lb_d[bi].unsqueeze(1))
        nc.sync.dma_start(g1s, d["g1_d"].rearrange("(a p) -> p a", p=128))
        nc.sync.dma_start(b1s, d["be1_d"].rearrange("(a p) -> p a", p=128))
        nc.sync.dma_start(g2s, d["g2_d"].rearrange("(a p) -> p a", p=128))
        nc.sync.dma_start(b2s, d["be2_d"].rearrange("(a p) -> p a", p=128))
        nc.sync.dma_start(bfc1s, d["bfc1_d"].rearrange("(a p) -> p a", p=128))
        nc.sync.dma_start(brow_f[:, 0, :], d["bproj_d"].unsqueeze(0))
        nc.sync.dma_start(brow_f[:, 1, :], d["bfc2_d"].unsqueeze(0))

        nc.vector.tensor_copy(wqkv8, wqkv_f)
        nc.vector.tensor_copy(wproj8, wproj_f)
        nc.vector.tensor_copy(wfc18, wfc1_f)
        nc.vector.tensor_copy(wfc28, wfc2_f)
        nc.vector.tensor_copy(lwb, lw_f)
        nc.vector.tensor_copy(bprow, brow_f[:, 0, :])
        nc.vector.tensor_copy(bf2row, brow_f[:, 1, :])
        id_f = stg.tile([128, 128], F32, name="id_f")
        make_identity(nc, id_f)
        nc.vector.tensor_copy(id_b, id_f)

        # input: token-major [112, 28, 256]; 4 DMAs for queue parallelism
        xsrc = x_d.rearrange("(i p) c -> p i c", p=TK)
        for j in range(4):
            nc.sync.dma_start(xtok[:, 7 * j:7 * j + 7, :], xsrc[:, 7 * j:7 * j + 7, :])

    # ---------------- LN (token-major) helper ----------------
    def layer_norm(src, mv, stdt, rstd, gs, bs, dst, lnp, tpp, apply_act):
        """src: [TK, NTOK, C] f32; dst: channel-major [128, 2, T] fp8 tile."""
        for i in range(NTOK):
            st = lnp.tile([TK, 6], F32, name="bnst", tag="bnst")
            nc.vector.bn_stats(st, src[:, i, :])
            nc.vector.bn_aggr(mv[:, i, :], st)
        nc.scalar.activation(stdt, mv[:, :, 1], AF.Sqrt, bias=eps_t)
        nc.vector.reciprocal(rstd, stdt)
        for g in range(7):
            lnt = []
            for j in range(4):
                i = 4 * g + j
                lt = lnp.tile([TK, C], BF16, name="lnt", tag="lnt")
                nc.vector.tensor_scalar(
                    out=lt, in0=src[:, i, :],
                    scalar1=mv[:, i, 0:1], scalar2=rstd[:, i:i + 1],
                    op0=OP.subtract, op1=OP.mult)
                lnt.append(lt)
            for c in range(2):
                tp = tpp.tile([128, 4 * TK], BF16, name="lntp", tag="lntp")
                for j in range(4):
                    nc.tensor.transpose(tp[:, TK * j:TK * (j + 1)],
                                        lnt[j][:, 128 * c:128 * (c + 1)],
                                        id_b[0:TK, 0:TK])
                if apply_act:
                    nc.scalar.activation(dst[:, c, 4 * TK * g:4 * TK * (g + 1)], tp,
                                         AF.Identity, bias=bs[:, c:c + 1],
                                         scale=gs[:, c:c + 1])
                else:
                    nc.vector.tensor_scalar(
                        out=dst[:, c, 4 * TK * g:4 * TK * (g + 1)], in0=tp,
                        scalar1=gs[:, c:c + 1], scalar2=bs[:, c:c + 1],
                        op0=OP.mult, op1=OP.add)

    # ---------------- attention-lifetime tensors ----------------
    actx = ExitStack()
    attn_pool = actx.enter_context(tc.tile_pool(name="attn_pool", bufs=1))
    ln1 = attn_pool.tile([128, 2, T], F8, name="ln1")
    qc = [attn_pool.tile([128, T], BF16, name=f"qc{b}") for b in range(2)]
    kc = [attn_pool.tile([128, T], BF16, name=f"kc{b}") for b in range(2)]
    vc = [attn_pool.tile([128, T], BF16, name=f"vc{b}") for b in range(2)]
    # vtb: [k-token, w, m, h, 36] fp8; col 32 of each 36-block is the ones col
    vtb = [attn_pool.tile([TK, NW, 4, NH, 36], F8, name=f"vtb{b}") for b in range(2)]
    lepe_sb = [attn_pool.tile([128, T], BF16, name=f"lepe{b}") for b in range(2)]
    att = attn_pool.tile([128, 2, T], F8, name="att")
    # double-buffered exp output: [k-token, m, h, q]
    etb = [attn_pool.tile([TK, 4, NH, L], F8, name=f"et{z}") for z in range(2)]
    dg = [attn_pool.tile([128, 9, 128], BF16, name=f"dg{b}") for b in range(2)]

    # zero-init (Pool, overlapped with loads): vtb + et m3 tail rows
    for b in range(2):
        nc.gpsimd.memset(vtb[b], 0.0)
    for z in range(2):
        nc.gpsimd.memset(etb[z][:, 3, :, :], 0.0)

    # Optional in-NEFF repetition loop for wall-clock timing (BASS_KERNEL_ITERS>1)
    import os as _os
    _iters = int(_os.environ.get("BASS_KERNEL_ITERS", "1"))
    loop_cm = tc.For_i(0, _iters, 1) if _iters > 1 else None
    if loop_cm is not None:
        ctx.enter_context(loop_cm)

    # ---------------- phase B: LN1 ----------------
    with tc.tile_pool(name="lnp1", bufs=8) as lnp, \
         tc.tile_pool(name="tpp1", bufs=2, space="PSUM") as tpp:
        layer_norm(xtok, mv1, std1, rstd1, g1s, b1s, ln1, lnp, tpp, True)

    # W-major views for branch 0 (stored idx = w*56 + h)
    qc0w = qc[0].rearrange("p (w h) -> p h w", w=WW)
    kc0w = kc[0].rearrange("p (w h) -> p h w", w=WW)
    vc0w = vc[0].rearrange("p (w h) -> p h w", w=WW)

    # ---------------- phase C: QKV + V prep + LePE ----------------
    with tc.tile_pool(name="qkvps", bufs=3, space="PSUM") as qkvps, \
         tc.tile_pool(name="vtps", bufs=2, space="PSUM") as vtps:
        # chunk name -> (branch, wqkv col offset, evac engine)
        chunks = [("k0", 0, 256, nc.scalar), ("q0", 0, 0, nc.scalar),
                  ("k1", 1, 384, nc.vector), ("q1", 1, 128, nc.vector),
                  ("v0", 0, 512, nc.scalar), ("v1", 1, 640, nc.vector)]
        wmaj = {"q0": qc0w, "k0": kc0w, "v0": vc0w}
        hmaj = {"q1": qc[1], "k1": kc[1], "v1": vc[1]}
        for ci, (nm, bi, c0, eng) in enumerate(chunks):
            for t2 in range(NW // 2):
                pt = qkvps.tile([128, 2, 512], F32, name="qkvt", tag="qkvt")
                for half in range(2):
                    t = 2 * t2 + half
                    nc.tensor.matmul(pt[:, half, 0:L],
                                     wqkv8[:, :, c0:c0 + 128],
                                     ln1[:, :, L * t:L * (t + 1)],
                                     start=True, stop=True,
                                     perf_mode=PM.DoubleRow)
                t0 = 2 * t2
                if nm in wmaj:
                    # H-major psum rows (h in [7t0..7t0+14)) scatter to W-major
                    dstw = wmaj[nm][:, 7 * t0:7 * t0 + 14, :].rearrange(
                        "p (a h) w -> p a h w", a=2)
                    src = pt[:, :, 0:L].rearrange("p a (x y) -> p a x y", x=7)
                    if eng is nc.scalar:
                        eng.copy(dstw, src)
                    else:
                        eng.tensor_copy(dstw, src)
                else:
                    dsth = hmaj[nm][:, L * t0:L * (t0 + 2)].rearrange(
                        "p (a x) -> p a x", a=2)
                    if eng is nc.scalar:
                        eng.copy(dsth, pt[:, :, 0:L])
                    else:
                        eng.tensor_copy(dsth, pt[:, :, 0:L])

        # V transposes -> vtb fp8 (token-major V with interleaved ones cols)
        for bi in range(2):
            for w in range(NW):
                vt = vtps.tile([TK, 4, 128], BF16, name="vtt", tag="vtt")
                for m in range(4):
                    mc = MCS[m]
                    nc.tensor.transpose(
                        vt[0:mc, m, :],
                        vc[bi][:, L * w + MCO[m]:L * w + MCO[m] + mc],
                        id_b)
                vtv = vt.rearrange("p m (h e) -> p m h e", h=NH)
                nc.vector.tensor_copy(vtb[bi][:, w, 0:3, :, 0:32], vtv[:, 0:3])
                nc.vector.tensor_copy(vtb[bi][0:56, w, 3, :, 0:32], vtv[0:56, 3])
                nc.vector.memset(vtb[bi][:, w, :, :, 32:33], 1.0)

        # LePE: 9 diag matmuls per window (ranged shifts, bf16)
        for bi in range(2):
            for tx in (-1, 0, 1):
                for ty in (-1, 0, 1):
                    wi = ((ty + 1) * 3 + (tx + 1)) if bi == 0 else ((tx + 1) * 3 + (ty + 1))
                    nc.gpsimd.affine_select(
                        out=dg[bi][:, wi, :],
                        in_=lwb[:, bi, wi:wi + 1].broadcast_to([128, 128]),
                        compare_op=OP.is_equal, fill=0.0, base=0,
                        pattern=[[-1, 128]], channel_multiplier=1)

    # ---------------- phase D: windowed attention ----------------
    att0w = att[:, 0, :].rearrange("p (w h) -> p h w", w=WW)
    with tc.tile_pool(name="dps", bufs=2, space="PSUM") as dps, \
         tc.tile_pool(name="onp", bufs=3) as onp, \
         tc.tile_pool(name="rqp", bufs=3) as rqp:
        taps = [(0, 0)] + [(tx, ty) for tx in (-1, 0, 1) for ty in (-1, 0, 1)
                           if (tx, ty) != (0, 0)]
        for bi in range(2):
            for w in range(NW):
                et = etb[(bi * NW + w) % 2]
                lp = dps.tile([128, L], F32, name="lpt", tag="dps")
                lpv = lp.rearrange("p (x y) -> p x y", x=7)
                vcv = vc[bi][:, L * w:L * (w + 1)].rearrange("p (x y) -> p x y", x=7)
                for ti, (tx, ty) in enumerate(taps):
                    wi = ((ty + 1) * 3 + (tx + 1)) if bi == 0 else ((tx + 1) * 3 + (ty + 1))
                    xo0, xo1 = max(0, -tx), 7 - max(0, tx)
                    yo0, yo1 = max(0, -ty), HH - max(0, ty)
                    nc.tensor.matmul(
                        lpv[:, xo0:xo1, yo0:yo1], dg[bi][:, wi, :],
                        vcv[:, xo0 + tx:xo1 + tx, yo0 + ty:yo1 + ty],
                        start=(ti == 0), stop=(ti == 8))
                nc.vector.tensor_scalar(
                    out=lepe_sb[bi][:, L * w:L * (w + 1)], in0=lp,
                    scalar1=lbs[:, bi:bi + 1], scalar2=None, op0=OP.add)
                for m in range(4):
                    mc = MCS[m]
                    sp = dps.tile([TK, 4, 512], F32, name="spst", tag="dps")
                    for h in range(NH):
                        nc.tensor.matmul(
                            sp[0:mc, h, 0:L],
                            kc[bi][32 * h:32 * (h + 1),
                                   L * w + MCO[m]:L * w + MCO[m] + mc],
                            qc[bi][32 * h:32 * (h + 1), L * w:L * (w + 1)],
                            start=True, stop=True, tile_position=(32 * h, 0))
                    nc.scalar.activation(
                        et[0:mc, m, :, :],
                        sp[0:mc, :, 0:L],
                        AF.Exp, scale=SM_SCALE)
                for q in range(4):
                    qn, qo = MCS[q], MCO[q]
                    ot = dps.tile([TK, NH, 34], F32, name="otst", tag="dps")
                    first = True
                    for j in range(2):
                        for h in range(NH):
                            nc.tensor.matmul(
                                ot[0:qn, h, 0:33],
                                et[:, 2 * j:2 * j + 2, h, qo:qo + qn],
                                vtb[bi][:, w, 2 * j:2 * j + 2, h, 0:33],
                                start=first, stop=(j == 1 and h == NH - 1),
                                perf_mode=PM.DoubleRow, skip_group_check=True)
                            first = False
                    rq = rqp.tile([TK, NH], F32, name="rq", tag="rq")
                    nc.vector.reciprocal(rq[0:qn], ot[0:qn, :, 32])
                    on = onp.tile([TK, NH * 32], BF16, name="on", tag="on")
                    onv = on.rearrange("p (a b) -> p a b", a=NH)
                    nc.vector.tensor_tensor(
                        out=onv[0:qn], in0=ot[0:qn, :, 0:32],
                        in1=rq[0:qn].unsqueeze(2).broadcast_to([qn, NH, 32]),
                        op=OP.mult)
                    ptp = dps.tile([128, TK], BF16, name="ptp", tag="dps")
                    nc.tensor.transpose(ptp[:, 0:qn], on[0:qn, :],
                                        id_b[0:qn, 0:qn])
                    lep = lepe_sb[bi][:, L * w + qo:L * w + qo + qn]
                    if bi == 0:
                        dst = att0w[:, :, 7 * w + qo // 56:7 * w + (qo + qn) // 56]
                        nc.vector.tensor_tensor(
                            out=dst,
                            in0=ptp[:, 0:qn].rearrange("p (a b) -> p b a", b=HH),
                            in1=lep.rearrange("p (a b) -> p b a", b=HH),
                            op=OP.add)
                    else:
                        nc.vector.tensor_tensor(
                            out=att[:, 1, L * w + qo:L * w + qo + qn],
                            in0=ptp[:, 0:qn], in1=lep, op=OP.add)

    # ---------------- phase E: proj + residual (token-major out) ----------------
    with tc.tile_pool(name="prps", bufs=4, space="PSUM") as prps:
        for i2 in range(NTOK // 2):
            pt = prps.tile([TK, 2, C], F32, name="prt", tag="prt")
            for half in range(2):
                i = 2 * i2 + half
                nc.tensor.matmul(pt[:, half, :], ones1, bprow,
                                 start=(half == 0), stop=False,
                                 skip_group_check=True)
                nc.tensor.matmul(pt[:, half, :],
                                 att[:, :, TK * i:TK * (i + 1)],
                                 wproj8,
                                 start=False, stop=(half == 1),
                                 perf_mode=PM.DoubleRow, skip_group_check=True)
            nc.vector.scalar_tensor_tensor(
                out=x2res[:, 2 * i2:2 * i2 + 2, :], in0=pt, scalar=1.0,
                in1=xtok[:, 2 * i2:2 * i2 + 2, :], op0=OP.mult, op1=OP.add)

    actx.close()

    # ---------------- phase F: LN2 + MLP ----------------
    with tc.tile_pool(name="mlp", bufs=1) as mlp:
        ln2 = mlp.tile([128, 2, T], F8, name="ln2")
        h_sb = mlp.tile([128, 8, T], F8, name="h_sb")
        with tc.tile_pool(name="lnp2", bufs=8) as lnp2, \
             tc.tile_pool(name="tpp2", bufs=2, space="PSUM") as tpp2:
            layer_norm(x2res, mv2, std2, rstd2, g2s, b2s, ln2, lnp2, tpp2, False)

        with tc.tile_pool(name="f1ps", bufs=2, space="PSUM") as f1ps, \
             tc.tile_pool(name="f2ps", bufs=2, space="PSUM") as f2ps, \
             tc.tile_pool(name="otp", bufs=4) as otp:
            for m8 in range(8):
                for tp2 in range(NW // 2):
                    pt = f1ps.tile([128, 2, 512], F32, name="f1t", tag="f1t")
                    for half in range(2):
                        t = 2 * tp2 + half
                        nc.tensor.matmul(pt[:, half, 0:L],
                                         wfc18[:, :, 128 * m8:128 * (m8 + 1)],
                                         ln2[:, :, L * t:L * (t + 1)],
                                         start=True, stop=True,
                                         perf_mode=PM.DoubleRow)
                    nc.scalar.activation(
                        h_sb[:, m8, 2 * L * tp2:2 * L * (tp2 + 1)].rearrange(
                            "p (a x) -> p a x", a=2),
                        pt[:, :, 0:L],
                        AF.Gelu, bias=bfc1s[:, m8:m8 + 1])

            for i2 in range(NTOK // 2):
                pt = f2ps.tile([TK, 2, C], F32, name="f2t", tag="f2t")
                for half in range(2):
                    i = 2 * i2 + half
                    nc.tensor.matmul(pt[:, half, :], ones1, bf2row,
                                     start=(half == 0), stop=False,
                                     skip_group_check=True)
                    for j in range(4):
                        nc.tensor.matmul(pt[:, half, :],
                                         h_sb[:, 2 * j:2 * j + 2, TK * i:TK * (i + 1)],
                                         wfc28[:, 2 * j:2 * j + 2, :],
                                         start=False,
                                         stop=(half == 1 and j == 3),
                                         perf_mode=PM.DoubleRow,
                                         skip_group_check=True)
                ot = otp.tile([TK, 2, C], F32, name="ot", tag="ot")
                nc.vector.scalar_tensor_tensor(
                    out=ot, in0=pt, scalar=1.0, in1=x2res[:, 2 * i2:2 * i2 + 2, :],
                    op0=OP.mult, op1=OP.add)
                eng = nc.sync if i2 % 2 == 0 else nc.scalar
                eng.dma_start(
                    out_d[2 * TK * i2:2 * TK * (i2 + 1), :].rearrange(
                        "(a p) c -> p a c", p=TK),
                    ot)


def kernel(**inputs):
    if "nc" not in _CACHE:
        _CACHE["nc"] = _build()
    nc = _CACHE["nc"]

    x = np.asarray(inputs["x"], dtype=np.float32)          # [8, 56, 56, 256]
    base = {
        "w_qkv": np.asarray(inputs["w_qkv"], np.float32),
        "w_proj": np.asarray(inputs["w_proj"], np.float32),
        "b_proj": np.asarray(inputs["b_proj"], np.float32),
        "gamma1": np.asarray(inputs["gamma1"], np.float32),
        "beta1": np.asarray(inputs["beta1"], np.float32),
        "gamma2": np.asarray(inputs["gamma2"], np.float32),
        "beta2": np.asarray(inputs["beta2"], np.float32),
        "w_fc1": np.asarray(inputs["w_fc1"], np.float32),
        "b_fc1": np.asarray(inputs["b_fc1"], np.float32),
        "w_fc2": np.asarray(inputs["w_fc2"], np.float32),
        "b_fc2": np.asarray(inputs["b_fc2"], np.float32),
        "lepe_w0": np.asarray(inputs["lepe_w0"], np.float32).reshape(128, 9),
        "lepe_w1": np.asarray(inputs["lepe_w1"], np.float32).reshape(128, 9),
        "lepe_b0": np.asarray(inputs["lepe_b0"], np.float32),
        "lepe_b1": np.asarray(inputs["lepe_b1"], np.float32),
    }
    in_maps = [{**base, "x": np.ascontiguousarray(x[i].reshape(T, C))}
               for i in range(B)]
    import os
    trace = bool(int(os.environ.get("BASS_KERNEL_TRACE", "0")))
    res = run_bass_kernel_spmd(nc, in_maps, core_ids=list(range(B)), trace=trace)
    _CACHE["last_results"] = res
    out = np.stack([res.results[i]["out"] for i in range(B)])
    return out.reshape(B, HH, WW, C)


if __name__ == "__main__":
    rng = np.random.default_rng(0)
    ins = {
        "x": rng.standard_normal((B, HH, WW, C), dtype=np.float32),
        "gamma1": np.ones(C, np.float32), "beta1": np.zeros(C, np.float32),
        "w_qkv": rng.standard_normal((C, 3 * C), dtype=np.float32) * 0.02,
        "lepe_w0": rng.standard_normal((128, 1, 3, 3), dtype=np.float32) * 0.02,
        "lepe_b0": np.zeros(128, np.float32),
        "lepe_w1": rng.standard_normal((128, 1, 3, 3), dtype=np.float32) * 0.02,
        "lepe_b1": np.zeros(128, np.float32),
        "w_proj": rng.standard_normal((C, C), dtype=np.float32) * 0.02,
        "b_proj": np.zeros(C, np.float32),
        "gamma2": np.ones(C, np.float32), "beta2": np.zeros(C, np.float32),
        "w_fc1": rng.standard_normal((C, 4 * C), dtype=np.float32) * 0.02,
        "b_fc1": np.zeros(4 * C, np.float32),
        "w_fc2": rng.standard_normal((4 * C, C), dtype=np.float32) * 0.02,
        "b_fc2": np.zeros(C, np.float32),
    }
    o = kernel(**ins)
    print("ran:", o.shape, o.dtype, float(np.abs(o).max()))


# revision 12
# speedup vs baseline: 1.3951x; 1.3693x over previous
"""CSWin block (B=8,H=W=56,C=256) on 8 trn2 NeuronCores, data-parallel over batch.

v2 layout strategy (per core, one image of 3136 tokens):
  - Residual stream token-major [112, 28, 256] fp32; LN stats token-major,
    LN output channel-major fp8 [128, 2(kch), T] feeding DoubleRow matmuls.
  - fp8e4 DoubleRow (0.5 cyc/output-row, 256-deep contraction per instruction)
    for QKV, fc1, fc2, proj. Biases for proj/fc2 enter PSUM via K=1 ones
    matmuls. b_fc1 rides the GELU activation per-partition.
  - Attention O is computed TRANSPOSED (window q-tokens on partitions) via
    fp8 DoubleRow over k-chunk pairs; a ones-column in V^T produces the
    softmax denominator as a per-partition column -> normalize is a
    per-partition reciprocal + stride-0-broadcast multiply, then one small
    PE transpose per q-chunk returns to channel-major for proj.
  - Branch-0 q/k/v stored W-major so its stripe windows are contiguous;
    PSUM evacuations scatter between H-major and W-major orders.
  - LePE depthwise conv stays on the PE as 9 diag-matmul taps (bf16).
  - exp on the Act engine is the wall (~95us); everything else is spread
    across DVE/Pool/PE underneath it.
"""

import sys

sys.path.insert(0, "/opt/trn_rl_repo")

import numpy as np
from contextlib import ExitStack

import concourse.bacc as bacc
import concourse.tile as tile
import concourse.mybir as mybir
from concourse.bass_utils import run_bass_kernel_spmd
from concourse.masks import make_identity

F32 = mybir.dt.float32
BF16 = mybir.dt.bfloat16
F8 = mybir.dt.float8e4
AF = mybir.ActivationFunctionType
OP = mybir.AluOpType
PM = mybir.MatmulPerfMode

B, HH, WW, C = 8, 56, 56, 256
T = HH * WW              # 3136 tokens
NW = 8                   # windows per branch
L = 392                  # tokens per window
NH = 4                   # heads per branch
HD = 32                  # head dim
TK = 112                 # token chunk for token-major phases
NTOK = T // TK           # 28
MCS = [112, 112, 112, 56]  # window k/q chunk sizes (112*3 + 56 = 392)
MCO = [0, 112, 224, 336]   # their offsets
EPS = 1e-5
SM_SCALE = float(HD) ** -0.5

_CACHE = {}


def _build():
    nc = bacc.Bacc("TRN2", target_bir_lowering=False, debug=False,
                   enable_asserts=False, num_devices=8)

    x_d = nc.dram_tensor("x", [T, C], F32, kind="ExternalInput").ap()
    out_d = nc.dram_tensor("out", [T, C], F32, kind="ExternalOutput").ap()
    wqkv_d = nc.dram_tensor("w_qkv", [C, 3 * C], F32, kind="ExternalInput").ap()
    wproj_d = nc.dram_tensor("w_proj", [C, C], F32, kind="ExternalInput").ap()
    bproj_d = nc.dram_tensor("b_proj", [C], F32, kind="ExternalInput").ap()
    g1_d = nc.dram_tensor("gamma1", [C], F32, kind="ExternalInput").ap()
    be1_d = nc.dram_tensor("beta1", [C], F32, kind="ExternalInput").ap()
    g2_d = nc.dram_tensor("gamma2", [C], F32, kind="ExternalInput").ap()
    be2_d = nc.dram_tensor("beta2", [C], F32, kind="ExternalInput").ap()
    wfc1_d = nc.dram_tensor("w_fc1", [C, 4 * C], F32, kind="ExternalInput").ap()
    bfc1_d = nc.dram_tensor("b_fc1", [4 * C], F32, kind="ExternalInput").ap()
    wfc2_d = nc.dram_tensor("w_fc2", [4 * C, C], F32, kind="ExternalInput").ap()
    bfc2_d = nc.dram_tensor("b_fc2", [C], F32, kind="ExternalInput").ap()
    lw_d = [nc.dram_tensor("lepe_w0", [128, 9], F32, kind="ExternalInput").ap(),
            nc.dram_tensor("lepe_w1", [128, 9], F32, kind="ExternalInput").ap()]
    lb_d = [nc.dram_tensor("lepe_b0", [128], F32, kind="ExternalInput").ap(),
            nc.dram_tensor("lepe_b1", [128], F32, kind="ExternalInput").ap()]

    with tile.TileContext(nc) as tc:
        with ExitStack() as ctx:
            _emit(nc, tc, ctx, locals())
    nc.compile()
    return nc


def _emit(nc, tc, ctx, d):
    x_d, out_d = d["x_d"], d["out_d"]
    lw_d, lb_d = d["lw_d"], d["lb_d"]

    pp = ctx.enter_context(tc.tile_pool(name="pp", bufs=1))
    psmall = ctx.enter_context(tc.tile_pool(name="psmall", bufs=1))

    # ---------------- persistent tensors ----------------
    xtok = pp.tile([TK, NTOK, C], F32, name="xtok")
    x2res = pp.tile([TK, NTOK, C], F32, name="x2res")
    wqkv8 = pp.tile([128, 2, 3 * C], F8, name="wqkv8")
    wproj8 = pp.tile([128, 2, C], F8, name="wproj8")
    wfc18 = pp.tile([128, 2, 4 * C], F8, name="wfc18")
    wfc28 = pp.tile([128, 8, C], F8, name="wfc28")
    lwb = pp.tile([128, 2, 9], BF16, name="lwb")
    g1s = psmall.tile([128, 2], F32, name="g1s")
    b1s = psmall.tile([128, 2], F32, name="b1s")
    g2s = psmall.tile([128, 2], F32, name="g2s")
    b2s = psmall.tile([128, 2], F32, name="b2s")
    bfc1s = psmall.tile([128, 8], F32, name="bfc1s")
    lbs = psmall.tile([128, 2], F32, name="lbs")
    bprow = psmall.tile([1, C], BF16, name="bprow")
    bf2row = psmall.tile([1, C], BF16, name="bf2row")
    ones1 = psmall.tile([1, TK], BF16, name="ones1")
    id_b = psmall.tile([128, 128], BF16, name="id_b")
    mv1 = psmall.tile([TK, NTOK, 2], F32, name="mv1")
    rstd1 = psmall.tile([TK, NTOK], F32, name="rstd1")
    std1 = psmall.tile([TK, NTOK], F32, name="std1")
    mv2 = psmall.tile([TK, NTOK, 2], F32, name="mv2")
    rstd2 = psmall.tile([TK, NTOK], F32, name="rstd2")
    std2 = psmall.tile([TK, NTOK], F32, name="std2")
    eps_t = psmall.tile([TK, 1], F32, name="eps_t")
    nc.vector.memset(eps_t, EPS)
    nc.vector.memset(ones1, 1.0)

    # ---------------- phase A: loads + weight conversion ----------------
    with tc.tile_pool(name="stg", bufs=1) as stg:
        wqkv_f = stg.tile([128, 2, 3 * C], F32, name="wqkv_f")
        wproj_f = stg.tile([128, 2, C], F32, name="wproj_f")
        wfc1_f = stg.tile([128, 2, 4 * C], F32, name="wfc1_f")
        wfc2_f = stg.tile([128, 8, C], F32, name="wfc2_f")
        lw_f = stg.tile([128, 2, 9], F32, name="lw_f")
        brow_f = stg.tile([1, 2, C], F32, name="brow_f")

        nc.sync.dma_start(wqkv_f, d["wqkv_d"].rearrange("(a p) n -> p a n", p=128))
        nc.sync.dma_start(wproj_f, d["wproj_d"].rearrange("(a p) n -> p a n", p=128))
        nc.sync.dma_start(wfc1_f, d["wfc1_d"].rearrange("(a p) n -> p a n", p=128))
        nc.sync.dma_start(wfc2_f, d["wfc2_d"].rearrange("(a p) n -> p a n", p=128))
        for bi in range(2):
            nc.sync.dma_start(lw_f[:, bi, :], lw_d[bi])
            nc.sync.dma_start(lbs[:, bi:bi + 1], lb_d[bi].unsqueeze(1))
        nc.sync.dma_start(g1s, d["g1_d"].rearrange("(a p) -> p a", p=128))
        nc.sync.dma_start(b1s, d["be1_d"].rearrange("(a p) -> p a", p=128))
        nc.sync.dma_start(g2s, d["g2_d"].rearrange("(a p) -> p a", p=128))
        nc.sync.dma_start(b2s, d["be2_d"].rearrange("(a p) -> p a", p=128))
        nc.sync.dma_start(bfc1s, d["bfc1_d"].rearrange("(a p) -> p a", p=128))
        nc.sync.dma_start(brow_f[:, 0, :], d["bproj_d"].unsqueeze(0))
        nc.sync.dma_start(brow_f[:, 1, :], d["bfc2_d"].unsqueeze(0))

        nc.vector.tensor_copy(wqkv8, wqkv_f)
        nc.vector.tensor_copy(wproj8, wproj_f)
        nc.vector.tensor_copy(wfc18, wfc1_f)
        nc.vector.tensor_copy(wfc28, wfc2_f)
        nc.vector.tensor_copy(lwb, lw_f)
        nc.vector.tensor_copy(bprow, brow_f[:, 0, :])
        nc.vector.tensor_copy(bf2row, brow_f[:, 1, :])
        id_f = stg.tile([128, 128], F32, name="id_f")
        make_identity(nc, id_f)
        nc.vector.tensor_copy(id_b, id_f)

        # input: token-major [112, 28, 256]; 4 DMAs for queue parallelism
        xsrc = x_d.rearrange("(i p) c -> p i c", p=TK)
        for j in range(4):
            nc.sync.dma_start(xtok[:, 7 * j:7 * j + 7, :], xsrc[:, 7 * j:7 * j + 7, :])

    # ---------------- LN (token-major) helper ----------------
    def layer_norm(src, mv, stdt, rstd, gs, bs, dst, lnp, tpp, apply_act,
                   stats=True):
        """src: [TK, NTOK, C] f32; dst: channel-major [128, 2, T] fp8 tile."""
        if stats:
            for i in range(NTOK):
                st = lnp.tile([TK, 6], F32, name="bnst", tag="bnst")
                nc.vector.bn_stats(st, src[:, i, :])
                nc.vector.bn_aggr(mv[:, i, :], st)
        nc.scalar.activation(stdt, mv[:, :, 1], AF.Sqrt, bias=eps_t)
        nc.vector.reciprocal(rstd, stdt)
        for g in range(7):
            lnt = []
            for j in range(4):
                i = 4 * g + j
                lt = lnp.tile([TK, C], BF16, name="lnt", tag="lnt")
                nc.vector.tensor_scalar(
                    out=lt, in0=src[:, i, :],
                    scalar1=mv[:, i, 0:1], scalar2=rstd[:, i:i + 1],
                    op0=OP.subtract, op1=OP.mult)
                lnt.append(lt)
            for c in range(2):
                tp = tpp.tile([128, 4 * TK], BF16, name="lntp", tag="lntp")
                for j in range(4):
                    nc.tensor.transpose(tp[:, TK * j:TK * (j + 1)],
                                        lnt[j][:, 128 * c:128 * (c + 1)],
                                        id_b[0:TK, 0:TK])
                if apply_act:
                    nc.scalar.activation(dst[:, c, 4 * TK * g:4 * TK * (g + 1)], tp,
                                         AF.Identity, bias=bs[:, c:c + 1],
                                         scale=gs[:, c:c + 1])
                else:
                    nc.vector.tensor_scalar(
                        out=dst[:, c, 4 * TK * g:4 * TK * (g + 1)], in0=tp,
                        scalar1=gs[:, c:c + 1], scalar2=bs[:, c:c + 1],
                        op0=OP.mult, op1=OP.add)

    # ---------------- attention-lifetime tensors ----------------
    actx = ExitStack()
    attn_pool = actx.enter_context(tc.tile_pool(name="attn_pool", bufs=1))
    ln1 = attn_pool.tile([128, 2, T], F8, name="ln1")
    qc = [attn_pool.tile([128, T], BF16, name=f"qc{b}") for b in range(2)]
    kc = [attn_pool.tile([128, T], BF16, name=f"kc{b}") for b in range(2)]
    vc = [attn_pool.tile([128, T], BF16, name=f"vc{b}") for b in range(2)]
    # vtb: [k-token, w, m, h, 36] fp8; col 32 of each 36-block is the ones col
    vtb = [attn_pool.tile([TK, NW, 4, NH, 36], F8, name=f"vtb{b}") for b in range(2)]
    lepe_sb = [attn_pool.tile([128, T], BF16, name=f"lepe{b}") for b in range(2)]
    att = attn_pool.tile([128, 2, T], F8, name="att")
    # double-buffered exp output: [k-token, m, h, q]
    etb = [attn_pool.tile([TK, 4, NH, L], F8, name=f"et{z}") for z in range(2)]
    dg = [attn_pool.tile([128, 9, 128], BF16, name=f"dg{b}") for b in range(2)]

    # zero-init (Pool, overlapped with loads): vtb + et m3 tail rows
    for b in range(2):
        nc.gpsimd.memset(vtb[b], 0.0)
    for z in range(2):
        nc.gpsimd.memset(etb[z][:, 3, :, :], 0.0)

    # Optional in-NEFF repetition loop for wall-clock timing (BASS_KERNEL_ITERS>1)
    import os as _os
    _iters = int(_os.environ.get("BASS_KERNEL_ITERS", "1"))
    loop_cm = tc.For_i(0, _iters, 1) if _iters > 1 else None
    if loop_cm is not None:
        ctx.enter_context(loop_cm)

    # ---------------- phase B: LN1 ----------------
    with tc.tile_pool(name="lnp1", bufs=8) as lnp, \
         tc.tile_pool(name="tpp1", bufs=2, space="PSUM") as tpp:
        layer_norm(xtok, mv1, std1, rstd1, g1s, b1s, ln1, lnp, tpp, True)

    # W-major views for branch 0 (stored idx = w*56 + h)
    qc0w = qc[0].rearrange("p (w h) -> p h w", w=WW)
    kc0w = kc[0].rearrange("p (w h) -> p h w", w=WW)
    vc0w = vc[0].rearrange("p (w h) -> p h w", w=WW)

    # ---------------- phase C: QKV + V prep + LePE ----------------
    with tc.tile_pool(name="qkvps", bufs=3, space="PSUM") as qkvps, \
         tc.tile_pool(name="vtps", bufs=2, space="PSUM") as vtps:
        # chunk name -> (branch, wqkv col offset, evac engine)
        chunks = [("k0", 0, 256, nc.scalar), ("q0", 0, 0, nc.vector),
                  ("v0", 0, 512, nc.scalar), ("k1", 1, 384, nc.vector),
                  ("q1", 1, 128, nc.scalar), ("v1", 1, 640, nc.vector)]
        wmaj = {"q0": qc0w, "k0": kc0w, "v0": vc0w}
        hmaj = {"q1": qc[1], "k1": kc[1], "v1": vc[1]}
        for ci, (nm, bi, c0, eng) in enumerate(chunks):
            for t2 in range(NW // 2):
                pt = qkvps.tile([128, 2, 512], F32, name="qkvt", tag="qkvt")
                for half in range(2):
                    t = 2 * t2 + half
                    nc.tensor.matmul(pt[:, half, 0:L],
                                     wqkv8[:, :, c0:c0 + 128],
                                     ln1[:, :, L * t:L * (t + 1)],
                                     start=True, stop=True,
                                     perf_mode=PM.DoubleRow)
                t0 = 2 * t2
                if nm in wmaj:
                    # H-major psum rows (h in [7t0..7t0+14)) scatter to W-major
                    dstw = wmaj[nm][:, 7 * t0:7 * t0 + 14, :].rearrange(
                        "p (a h) w -> p a h w", a=2)
                    src = pt[:, :, 0:L].rearrange("p a (x y) -> p a x y", x=7)
                    if eng is nc.scalar:
                        eng.copy(dstw, src)
                    else:
                        eng.tensor_copy(dstw, src)
                else:
                    dsth = hmaj[nm][:, L * t0:L * (t0 + 2)].rearrange(
                        "p (a x) -> p a x", a=2)
                    if eng is nc.scalar:
                        eng.copy(dsth, pt[:, :, 0:L])
                    else:
                        eng.tensor_copy(dsth, pt[:, :, 0:L])

        # V transposes -> vtb fp8 (token-major V with interleaved ones cols)
        for bi in range(2):
            for w in range(NW):
                vt = vtps.tile([TK, 4, 128], BF16, name="vtt", tag="vtt")
                for m in range(4):
                    mc = MCS[m]
                    nc.tensor.transpose(
                        vt[0:mc, m, :],
                        vc[bi][:, L * w + MCO[m]:L * w + MCO[m] + mc],
                        id_b)
                vtv = vt.rearrange("p m (h e) -> p m h e", h=NH)
                nc.vector.tensor_copy(vtb[bi][:, w, 0:3, :, 0:32], vtv[:, 0:3])
                nc.vector.tensor_copy(vtb[bi][0:56, w, 3, :, 0:32], vtv[0:56, 3])
                nc.vector.memset(vtb[bi][:, w, :, :, 32:33], 1.0)

        # LePE: 9 diag matmuls per window (ranged shifts, bf16)
        for bi in range(2):
            for tx in (-1, 0, 1):
                for ty in (-1, 0, 1):
                    wi = ((ty + 1) * 3 + (tx + 1)) if bi == 0 else ((tx + 1) * 3 + (ty + 1))
                    nc.gpsimd.affine_select(
                        out=dg[bi][:, wi, :],
                        in_=lwb[:, bi, wi:wi + 1].broadcast_to([128, 128]),
                        compare_op=OP.is_equal, fill=0.0, base=0,
                        pattern=[[-1, 128]], channel_multiplier=1)

    # ---------------- phase D: windowed attention ----------------
    # PSUM: sp tag = [128, 2, 512] f32 (2 banks) x2 bufs; aux tag = 1-bank
    # x4 bufs holding lp / otA / otB / ptp / prt. exp runs per (m, head-pair)
    # so S double-buffering never waits on the window tail.
    att0w = att[:, 0, :].rearrange("p (w h) -> p h w", w=WW)
    taps = [(0, 0)] + [(tx, ty) for tx in (-1, 0, 1) for ty in (-1, 0, 1)
                       if (tx, ty) != (0, 0)]

    def emit_proj(i2, pool):
        pt = pool.tile([TK, 2, C], F32, name="prt", tag="aux", bufs=4)
        for half in range(2):
            i = 2 * i2 + half
            nc.tensor.matmul(pt[:, half, :], ones1, bprow,
                             start=(half == 0), stop=False,
                             skip_group_check=True)
            nc.tensor.matmul(pt[:, half, :],
                             att[:, :, TK * i:TK * (i + 1)],
                             wproj8,
                             start=False, stop=(half == 1),
                             perf_mode=PM.DoubleRow, skip_group_check=True)
        nc.vector.scalar_tensor_tensor(
            out=x2res[:, 2 * i2:2 * i2 + 2, :], in0=pt, scalar=1.0,
            in1=xtok[:, 2 * i2:2 * i2 + 2, :], op0=OP.mult, op1=OP.add)
        # LN2 stats for these two token tiles (DVE, no PSUM)
        for i in (2 * i2, 2 * i2 + 1):
            st = onp.tile([TK, 6], F32, name="bnst2", tag="bnst2")
            nc.vector.bn_stats(st, x2res[:, i, :])
            nc.vector.bn_aggr(mv2[:, i, :], st)

    proj_done = 0
    with tc.tile_pool(name="dps", bufs=2, space="PSUM") as dps, \
         tc.tile_pool(name="onp", bufs=3) as onp, \
         tc.tile_pool(name="rqp", bufs=3) as rqp:
        for bi in range(2):
            for w in range(NW):
                et = etb[(bi * NW + w) % 2]
                vcv = vc[bi][:, L * w:L * (w + 1)].rearrange("p (x y) -> p x y", x=7)
                # LePE: 9 ranged diag-matmul taps -> lp (aux bank)
                lp = dps.tile([128, 512], F32, name="lpt", tag="aux", bufs=4)
                lpv = lp[:, 0:L].rearrange("p (x y) -> p x y", x=7)
                for ti, (tx, ty) in enumerate(taps):
                    wi = ((ty + 1) * 3 + (tx + 1)) if bi == 0 else ((tx + 1) * 3 + (ty + 1))
                    xo0, xo1 = max(0, -tx), 7 - max(0, tx)
                    yo0, yo1 = max(0, -ty), HH - max(0, ty)
                    nc.tensor.matmul(
                        lpv[:, xo0:xo1, yo0:yo1], dg[bi][:, wi, :],
                        vcv[:, xo0 + tx:xo1 + tx, yo0 + ty:yo1 + ty],
                        start=(ti == 0), stop=(ti == 8))
                nc.vector.tensor_scalar(
                    out=lepe_sb[bi][:, L * w:L * (w + 1)], in0=lp[:, 0:L],
                    scalar1=lbs[:, bi:bi + 1], scalar2=None, op0=OP.add)
                # S + exp per (m-chunk, head-pair)
                for m in range(4):
                    mc = MCS[m]
                    for hp in range(2):
                        sp = dps.tile([128, 2, 512], F32, name="spst", tag="sp")
                        for hh in range(2):
                            h = 2 * hp + hh
                            nc.tensor.matmul(
                                sp[0:mc, hh, 0:L],
                                kc[bi][32 * h:32 * (h + 1),
                                       L * w + MCO[m]:L * w + MCO[m] + mc],
                                qc[bi][32 * h:32 * (h + 1), L * w:L * (w + 1)],
                                start=True, stop=True, tile_position=(32 * h, 0))
                        nc.scalar.activation(
                            et[0:mc, m, 2 * hp:2 * hp + 2, :],
                            sp[0:mc, :, 0:L],
                            AF.Exp, scale=SM_SCALE)
                # O^T via fp8 DoubleRow (q-pairs share a 1-bank aux tile)
                ots = []
                for qp in range(2):
                    ot = dps.tile([TK, 2, NH, 34], F32, name="otst", tag="aux",
                                  bufs=4)
                    ots.append(ot)
                    first = True
                    for qq in range(2):
                        q = 2 * qp + qq
                        qn = MCS[q]
                        for j in range(2):
                            for h in range(NH):
                                nc.tensor.matmul(
                                    ot[0:qn, qq, h, 0:33],
                                    et[:, 2 * j:2 * j + 2, h, MCO[q]:MCO[q] + qn],
                                    vtb[bi][:, w, 2 * j:2 * j + 2, h, 0:33],
                                    start=first,
                                    stop=(qq == 1 and j == 1 and h == NH - 1),
                                    perf_mode=PM.DoubleRow, skip_group_check=True)
                                first = False
                ptp = dps.tile([128, NH, TK], BF16, name="ptp", tag="aux", bufs=4)
                for q in range(4):
                    qn, qo = MCS[q], MCO[q]
                    ot = ots[q // 2]
                    rq = rqp.tile([TK, NH], F32, name="rq", tag="rq")
                    nc.vector.reciprocal(rq[0:qn], ot[0:qn, q % 2, :, 32])
                    on = onp.tile([TK, NH * 32], BF16, name="on", tag="on")
                    onv = on.rearrange("p (a b) -> p a b", a=NH)
                    nc.vector.tensor_tensor(
                        out=onv[0:qn], in0=ot[0:qn, q % 2, :, 0:32],
                        in1=rq[0:qn].unsqueeze(2).broadcast_to([qn, NH, 32]),
                        op=OP.mult)
                    nc.tensor.transpose(ptp[:, q, 0:qn], on[0:qn, :],
                                        id_b[0:qn, 0:qn])
                    lep = lepe_sb[bi][:, L * w + qo:L * w + qo + qn]
                    if bi == 0:
                        dst = att0w[:, :, 7 * w + qo // 56:7 * w + (qo + qn) // 56]
                        nc.vector.tensor_tensor(
                            out=dst,
                            in0=ptp[:, q, 0:qn].rearrange("p (a b) -> p b a", b=HH),
                            in1=lep.rearrange("p (a b) -> p b a", b=HH),
                            op=OP.add)
                    else:
                        nc.vector.tensor_tensor(
                            out=att[:, 1, L * w + qo:L * w + qo + qn],
                            in0=ptp[:, q, 0:qn], in1=lep, op=OP.add)
                if bi == 1:
                    while 224 * (proj_done + 1) <= 392 * (w + 1):
                        emit_proj(proj_done, dps)
                        proj_done += 1

    # ---------------- phase E: proj + residual (token-major out) ----------------
    actx.close()

    # ---------------- phase F: LN2 + MLP ----------------
    with tc.tile_pool(name="mlp", bufs=1) as mlp:
        ln2 = mlp.tile([128, 2, T], F8, name="ln2")
        h_sb = mlp.tile([128, 8, T], F8, name="h_sb")
        with tc.tile_pool(name="lnp2", bufs=8) as lnp2, \
             tc.tile_pool(name="tpp2", bufs=2, space="PSUM") as tpp2:
            layer_norm(x2res, mv2, std2, rstd2, g2s, b2s, ln2, lnp2, tpp2, True,
                       stats=False)

        with tc.tile_pool(name="f1ps", bufs=2, space="PSUM") as f1ps, \
             tc.tile_pool(name="f2ps", bufs=2, space="PSUM") as f2ps, \
             tc.tile_pool(name="otp", bufs=4) as otp:
            for m8 in range(8):
                for tp2 in range(NW // 2):
                    pt = f1ps.tile([128, 2, 512], F32, name="f1t", tag="f1t")
                    for half in range(2):
                        t = 2 * tp2 + half
                        nc.tensor.matmul(pt[:, half, 0:L],
                                         wfc18[:, :, 128 * m8:128 * (m8 + 1)],
                                         ln2[:, :, L * t:L * (t + 1)],
                                         start=True, stop=True,
                                         perf_mode=PM.DoubleRow)
                    nc.scalar.activation(
                        h_sb[:, m8, 2 * L * tp2:2 * L * (tp2 + 1)].rearrange(
                            "p (a x) -> p a x", a=2),
                        pt[:, :, 0:L],
                        AF.Gelu, bias=bfc1s[:, m8:m8 + 1])

            for i2 in range(NTOK // 2):
                pt = f2ps.tile([TK, 2, C], F32, name="f2t", tag="f2t")
                for half in range(2):
                    i = 2 * i2 + half
                    nc.tensor.matmul(pt[:, half, :], ones1, bf2row,
                                     start=(half == 0), stop=False,
                                     skip_group_check=True)
                    for j in range(4):
                        nc.tensor.matmul(pt[:, half, :],
                                         h_sb[:, 2 * j:2 * j + 2, TK * i:TK * (i + 1)],
                                         wfc28[:, 2 * j:2 * j + 2, :],
                                         start=False,
                                         stop=(half == 1 and j == 3),
                                         perf_mode=PM.DoubleRow,
                                         skip_group_check=True)
                ot = otp.tile([TK, 2, C], F32, name="ot", tag="ot")
                nc.vector.scalar_tensor_tensor(
                    out=ot, in0=pt, scalar=1.0, in1=x2res[:, 2 * i2:2 * i2 + 2, :],
                    op0=OP.mult, op1=OP.add)
                eng = nc.sync if i2 % 2 == 0 else nc.scalar
                eng.dma_start(
                    out_d[2 * TK * i2:2 * TK * (i2 + 1), :].rearrange(
                        "(a p) c -> p a c", p=TK),
                    ot)


def kernel(**inputs):
    if "nc" not in _CACHE:
        _CACHE["nc"] = _build()
    nc = _CACHE["nc"]

    x = np.asarray(inputs["x"], dtype=np.float32)          # [8, 56, 56, 256]
    base = {
        "w_qkv": np.asarray(inputs["w_qkv"], np.float32),
        "w_proj": np.asarray(inputs["w_proj"], np.float32),
        "b_proj": np.asarray(inputs["b_proj"], np.float32),
        "gamma1": np.asarray(inputs["gamma1"], np.float32),
        "beta1": np.asarray(inputs["beta1"], np.float32),
        "gamma2": np.asarray(inputs["gamma2"], np.float32),
        "beta2": np.asarray(inputs["beta2"], np.float32),
        "w_fc1": np.asarray(inputs["w_fc1"], np.float32),
        "b_fc1": np.asarray(inputs["b_fc1"], np.float32),
        "w_fc2": np.asarray(inputs["w_fc2"], np.float32),
        "b_fc2": np.asarray(inputs["b_fc2"], np.float32),
        "lepe_w0": np.asarray(inputs["lepe_w0"], np.float32).reshape(128, 9),
        "lepe_w1": np.asarray(inputs["lepe_w1"], np.float32).reshape(128, 9),
        "lepe_b0": np.asarray(inputs["lepe_b0"], np.float32),
        "lepe_b1": np.asarray(inputs["lepe_b1"], np.float32),
    }
    in_maps = [{**base, "x": np.ascontiguousarray(x[i].reshape(T, C))}
               for i in range(B)]
    import os
    trace = bool(int(os.environ.get("BASS_KERNEL_TRACE", "0")))
    res = run_bass_kernel_spmd(nc, in_maps, core_ids=list(range(B)), trace=trace)
    _CACHE["last_results"] = res
    out = np.stack([res.results[i]["out"] for i in range(B)])
    return out.reshape(B, HH, WW, C)


if __name__ == "__main__":
    rng = np.random.default_rng(0)
    ins = {
        "x": rng.standard_normal((B, HH, WW, C), dtype=np.float32),
        "gamma1": np.ones(C, np.float32), "beta1": np.zeros(C, np.float32),
        "w_qkv": rng.standard_normal((C, 3 * C), dtype=np.float32) * 0.02,
        "lepe_w0": rng.standard_normal((128, 1, 3, 3), dtype=np.float32) * 0.02,
        "lepe_b0": np.zeros(128, np.float32),
        "lepe_w1": rng.standard_normal((128, 1, 3, 3), dtype=np.float32) * 0.02,
        "lepe_b1": np.zeros(128, np.float32),
        "w_proj": rng.standard_normal((C, C), dtype=np.float32) * 0.02,
        "b_proj": np.zeros(C, np.float32),
        "gamma2": np.ones(C, np.float32), "beta2": np.zeros(C, np.float32),
        "w_fc1": rng.standard_normal((C, 4 * C), dtype=np.float32) * 0.02,
        "b_fc1": np.zeros(4 * C, np.float32),
        "w_fc2": rng.standard_normal((4 * C, C), dtype=np.float32) * 0.02,
        "b_fc2": np.zeros(C, np.float32),
    }
    o = kernel(**ins)
    print("ran:", o.shape, o.dtype, float(np.abs(o).max()))
